# revision 32
# baseline (speedup 1.0000x reference)
"""CogVLM vision-expert attention on 8 Trainium2 NeuronCores.

Sharding: tensor-parallel over heads (4 heads per core). Each core gets
- replicated: hidden_states (bf16, [128,2,32,1024] tiled), RoPE tables,
  packed binary key-mask blocks
- sharded:    QKV weight columns + dense weight rows for its 4 heads
Host sorts tokens language-first, so expert routing becomes an exact
column split at L (no dual-expert compute, no predicated selects).
Attention computes transposed scores s^T[j,i] per key-block, exps them
PSUM->SBUF, applies the 0/1 mask multiplicatively, row-sums via
ones-matmuls on the PE, and folds 1/sum into the PSUM->SBUF context
copy.  Everything on the matmul path is bf16 (f32 PSUM accumulate).
The host sums the 8 bf16 row-parallel dense partials.

Self-contained: hardcodes all shapes; only needs numpy + ml_dtypes +
concourse (on sys.path in this container).
"""

import numpy as np

B, S, H, NH = 1, 2048, 4096, 32
HD = H // NH          # 128
NCORES = 8
HPC = NH // NCORES    # 4 heads per core
NBLK = 3 * HPC        # 12 qkv col-blocks of 128 per core
NT = S // 128         # 16 token blocks
ROPE_BASE = 10000.0

_CACHE = {}


def _chunks(lo, hi, L):
    """Split [lo,hi) at multiples of 512 and at L -> [(c0,c1,expert)].
    expert 1 = language (tokens < L), 0 = vision."""
    pts = {lo, hi}
    pts.update(c for c in range(0, S + 1, 512) if lo < c < hi)
    if lo < L < hi:
        pts.add(L)
    pts = sorted(pts)
    return [(pts[i], pts[i + 1], 1 if pts[i + 1] <= L else 0)
            for i in range(len(pts) - 1)]


def _build(L, mask_info):
    import concourse.bass as bass
    import concourse.mybir as mybir
    import concourse.tile as tile
    from concourse import bacc
    from contextlib import ExitStack
    import ml_dtypes

    dt = mybir.dt
    f32, bf16 = dt.float32, dt.bfloat16
    AF = mybir.ActivationFunctionType

    # ---- derived compile-time structure from the mask ----
    # mask_info[it][jt]: 0 = all-allowed, 1 = mixed, 2 = all-masked
    mix_idx = {}
    for jt in range(NT):
        for it in range(NT):
            if mask_info[it][jt] == 1:
                mix_idx[(jt, it)] = len(mix_idx)
    NMIX = len(mix_idx)

    HNT = NT // 2
    # per (jt, i-half): valid-it segments (<=4 blocks) and mixed-it runs
    segs_by = {}      # (jt, ih) -> [(i0b, i1b)]
    mixruns_by = {}   # (jt, ih) -> [(a0b, a1b, mstart)]
    for jt in range(NT):
        for ih in range(2):
            lo, hi = ih * HNT, ih * HNT + HNT
            valid = [it for it in range(lo, hi) if mask_info[it][jt] != 2]
            segs = []
            k = 0
            while k < len(valid):
                j0 = k
                while (k + 1 < len(valid) and valid[k + 1] == valid[k] + 1
                       and k + 1 - j0 < 8):
                    k += 1
                segs.append((valid[j0], valid[k] + 1))
                k += 1
            segs_by[(jt, ih)] = segs
            runs = []
            k = 0
            mixed = [it for it in range(lo, hi) if mask_info[it][jt] == 1]
            while k < len(mixed):
                j0 = k
                while k + 1 < len(mixed) and mixed[k + 1] == mixed[k] + 1:
                    k += 1
                runs.append((mixed[j0], mixed[k] + 1,
                             mix_idx[(jt, mixed[j0])]))
                k += 1
            mixruns_by[(jt, ih)] = runs
    vjts_by_it = [[jt for jt in range(NT) if mask_info[it][jt] != 2]
                  for it in range(NT)]

    nc = bacc.Bacc("TRN2", target_bir_lowering=False, debug=False)

    hsT = nc.dram_tensor("hsT", [128, 2, 32, 1024], bf16, kind="ExternalInput")
    wqkv = nc.dram_tensor("wqkv", [2, NBLK, 128, 32, 128], bf16,
                          kind="ExternalInput")
    wdense = nc.dram_tensor("wdense", [32, 128, 2, HPC, 128], bf16,
                            kind="ExternalInput")
    cosT = nc.dram_tensor("cosT", [HD, S], bf16, kind="ExternalInput")
    sinT = nc.dram_tensor("sinT", [HD, S], bf16, kind="ExternalInput")
    mpk = nc.dram_tensor("mpk", [128, max(NMIX, 1), 128], bf16,
                         kind="ExternalInput")
    outT = nc.dram_tensor("outT", [32, 128, S], bf16, kind="ExternalOutput")

    eye_bf16 = nc.inline_tensor(np.eye(128, dtype=ml_dtypes.bfloat16),
                                "eye_bf16")
    RT_np = np.zeros((128, 128), dtype=np.float32)
    for j in range(64):
        RT_np[j, j + 64] = 1.0
        RT_np[j + 64, j] = -1.0
    RT_t = nc.inline_tensor(RT_np.astype(ml_dtypes.bfloat16), "RT")
    ones_t = nc.inline_tensor(np.ones((128, 1), dtype=ml_dtypes.bfloat16),
                              "ones")

    qkv_chunks = {th: _chunks(th * 1024, th * 1024 + 1024, L)
                  for th in range(2)}
    out_chunks = _chunks(0, S, L)

    with tile.TileContext(nc) as tc, ExitStack() as top:
        singles = top.enter_context(tc.tile_pool(name="singles", bufs=1))

        ident16 = singles.tile([128, 128], bf16)
        nc.gpsimd.dma_start(out=ident16, in_=eye_bf16[:, :])
        RT_sb = singles.tile([128, 128], bf16)
        nc.gpsimd.dma_start(out=RT_sb, in_=RT_t[:, :])
        ones_sb = singles.tile([128, 1], bf16)
        nc.gpsimd.dma_start(out=ones_sb, in_=ones_t[:, :])
        cos_sb = singles.tile([HD, S], bf16)
        nc.gpsimd.dma_start(out=cos_sb, in_=cosT[:, :])
        sin_sb = singles.tile([HD, S], bf16)
        nc.gpsimd.dma_start(out=sin_sb, in_=sinT[:, :])
        nbias = singles.tile([128, 1], f32)
        nc.vector.memset(nbias, -24.0)

        qkv_pool = top.enter_context(tc.tile_pool(name="qkv", bufs=1))
        qkv_sb = [qkv_pool.tile([128, S], bf16, tag="qkv", bufs=NBLK,
                                name=f"qkv_{nb}") for nb in range(NBLK)]
        ctx_pool = top.enter_context(tc.tile_pool(name="ctx", bufs=1))
        ctxT_sb = [ctx_pool.tile([128, S], bf16, tag="ctxT", bufs=HPC,
                                 name=f"ctxT_{hl}") for hl in range(HPC)]

        # ---------------- Stage A: column-split QKV projection ----------
        with ExitStack() as sa:
            pa = sa.enter_context(tc.tile_pool(name="qkv_sbuf", bufs=1))
            ppa = sa.enter_context(tc.tile_pool(name="qkv_psum", bufs=1,
                                                space="PSUM"))
            ncopy = 0
            for th in range(2):
                t0 = th * 1024
                experts = sorted({e for _, _, e in qkv_chunks[th]})
                hk = []
                for i in range(8):
                    hki = pa.tile([128, 4, 1024], bf16, tag="hsT", bufs=10,
                                  name=f"hsT_{th}_{i}")
                    nc.gpsimd.dma_start(out=hki,
                                        in_=hsT[:, th, 4 * i:4 * i + 4, :])
                    hk.append(hki)
                for nb in range(NBLK):
                    wb = {}
                    for e in experts:
                        wbe = pa.tile([128, 32, 128], bf16, tag=f"w{e}",
                                      bufs=3, name=f"w_{th}_{nb}_{e}")
                        nc.sync.dma_start(out=wbe, in_=wqkv[e, nb, :, :, :])
                        wb[e] = wbe
                    for c0, c1, e in qkv_chunks[th]:
                        w = c1 - c0
                        psf = ppa.tile([128, 512], f32, tag="ps", bufs=3,
                                       name=f"ps_{th}_{nb}_{c0}")
                        ps = psf[:, :w]
                        for kt in range(32):
                            nc.tensor.matmul(
                                ps,
                                lhsT=wb[e][:, kt, :],
                                rhs=hk[kt // 4][:, kt % 4,
                                                c0 - t0:c1 - t0],
                                start=(kt == 0), stop=(kt == 31),
                            )
                        dst = qkv_sb[nb][:, c0:c1]
                        if ncopy % 2 == 0:
                            nc.vector.tensor_copy(out=dst, in_=ps)
                        else:
                            nc.scalar.copy(out=dst, in_=ps)
                        ncopy += 1

        # ---------------- Stage B: per-head attention -------------------
        pc = top.enter_context(tc.tile_pool(name="dense_sbuf", bufs=1))
        wd_tiles = {}

        def load_wd(nb):
            wd = pc.tile([128, 2, HPC, 128], bf16, tag="wd", bufs=3,
                         name=f"wd_{nb}")
            nc.sync.dma_start(out=wd, in_=wdense[nb, :, :, :, :])
            wd_tiles[nb] = wd

        with ExitStack() as sb:
            pb = sb.enter_context(tc.tile_pool(name="att_sbuf", bufs=1))
            ppb = sb.enter_context(tc.tile_pool(name="att_psum", bufs=1,
                                                space="PSUM"))
            mq = pb.tile([128, max(NMIX, 1), 128], bf16, tag="mq", bufs=1)
            nc.gpsimd.dma_start(out=mq, in_=mpk[:, :, :])

            def rope(hl):
                # RoPE: x' = x*cos + (R @ x)*sin (scale folded into tables).
                # Returns per-chunk emitters so callers can interleave them
                # under other PE work (the DVE muls are the long pole).
                qr = pb.tile([128, S], bf16, tag="qr", bufs=2,
                             name=f"qr_{hl}")
                kr = pb.tile([128, S], bf16, tag="kr", bufs=2,
                             name=f"kr_{hl}")

                def chunk(xT, xr, ch):
                    def emit():
                        cs = slice(ch * 1024, ch * 1024 + 1024)
                        rps = ppb.tile([128, 1024], f32, tag="mm", bufs=3,
                                       name=f"rot_{hl}_{ch}")
                        for s in range(2):
                            nc.tensor.matmul(
                                rps[:, s * 512:s * 512 + 512],
                                lhsT=RT_sb,
                                rhs=xT[:, ch * 1024 + s * 512:
                                       ch * 1024 + s * 512 + 512],
                                start=True, stop=True)
                        t1 = pb.tile([128, 1024], bf16, tag="ropet", bufs=3,
                                     name=f"rt_{hl}_{ch}")
                        nc.vector.tensor_mul(out=t1, in0=rps,
                                             in1=sin_sb[:, cs])
                        nc.vector.tensor_mul(out=xr[:, cs], in0=xT[:, cs],
                                             in1=cos_sb[:, cs])
                        nc.vector.tensor_add(out=xr[:, cs], in0=xr[:, cs],
                                             in1=t1)
                    return emit

                emitters = [chunk(xT, xr, ch)
                            for xT, xr in ((qkv_sb[3 * hl], qr),
                                           (qkv_sb[3 * hl + 1], kr))
                            for ch in range(2)]
                return qr, kr, emitters

            def vtrans(hl):
                # 4 transposes packed per PSUM bank (start=True on the first
                # clears the whole bank), then one wide DVE copy each.
                v_sb = pb.tile([128, NT, 128], bf16, tag="vsb", bufs=2,
                               name=f"v_{hl}")
                vT = qkv_sb[3 * hl + 2]

                def group(g):
                    def emit():
                        vtpf = ppb.tile([128, 1024], f32, tag="mm", bufs=3,
                                        name=f"vt_{hl}_{g}")
                        vtp = vtpf.bitcast(bf16)[:, :512]
                        for k in range(4):
                            jt = 4 * g + k
                            nc.tensor.matmul(
                                vtp[:, k * 128:(k + 1) * 128],
                                lhsT=vT[:, jt * 128:(jt + 1) * 128],
                                rhs=ident16, is_transpose=True,
                                start=(k == 0), stop=(k == 3),
                                skip_group_check=True,
                            )
                        nc.vector.tensor_copy(
                            out=v_sb[:, 4 * g:4 * g + 4, :], in_=vtp)
                    return emit

                return v_sb, [group(g) for g in range(4)]

            def scores_half(hl, ih, qr, kr, pT):
                # per-jt closures: transposed scores + exp + mult. mask for
                # query blocks in i-half ih; results land in pT[(jt, ih)].
                def one(jt):
                    def emit():
                        segs = segs_by[(jt, ih)]
                        base = segs[0][0]
                        width = (segs[-1][1] - base) * 128
                        pt = pb.tile([128, width], bf16,
                                     tag=f"pT{jt}_{ih}", bufs=1,
                                     name=f"pT_{hl}_{jt}_{ih}")
                        pT[(jt, ih)] = (pt, base)
                        for i0b, i1b in segs:
                            w = (i1b - i0b) * 128
                            spf = ppb.tile([128, 1024], f32, tag="mm",
                                           bufs=3,
                                           name=f"sp_{hl}_{jt}_{i0b}")
                            sp = spf[:, :w]
                            for s in range(0, w, 512):
                                nc.tensor.matmul(
                                    sp[:, s:min(w, s + 512)],
                                    lhsT=kr[:, jt * 128:(jt + 1) * 128],
                                    rhs=qr[:, i0b * 128 + s:
                                           i0b * 128 + min(w, s + 512)],
                                    start=True, stop=True,
                                )
                            off = (i0b - base) * 128
                            # logits are O(10) at this input scale;
                            # exp(x-24) cannot overflow and softmax is
                            # shift-invariant.
                            nc.scalar.activation(
                                out=pt[:, off:off + w], in_=sp,
                                func=AF.Exp, bias=nbias, scale=1.0,
                            )
                        for a0, a1, m0 in mixruns_by[(jt, ih)]:
                            w = (a1 - a0) * 128
                            off = (a0 - base) * 128
                            nc.vector.tensor_mul(
                                out=pt[:, off:off + w],
                                in0=pt[:, off:off + w],
                                in1=mq[:, m0:m0 + (a1 - a0), :],
                            )
                    return emit

                return [one(jt) for jt in range(NT) if segs_by[(jt, ih)]]

            def pv_half(hl, ih, pT, v_sb, ctis, side=()):
                # per-query-block: denominators, PV, DVE scale; transposes
                # batched 4-per-bank, deferred one group so the chain hides
                # under the next group's PV matmuls.  `side` closures (next
                # half/head's scores + rope) are drained between blocks.
                side = list(side)

                def flush(g):
                    ctpf = ppb.tile([128, 1024], f32, tag="mm", bufs=3,
                                    name=f"ct_{hl}_{g}")
                    ctp = ctpf.bitcast(bf16)[:, :512]
                    for k in range(4):
                        nc.tensor.matmul(
                            ctp[:, k * 128:(k + 1) * 128],
                            lhsT=ctis[4 * g + k], rhs=ident16,
                            is_transpose=True,
                            start=(k == 0), stop=(k == 3),
                            skip_group_check=True,
                        )
                    nc.vector.tensor_copy(
                        out=ctxT_sb[hl][:, g * 512:(g + 1) * 512],
                        in_=ctp)

                per_it = -(-len(side) // HNT)
                for it in range(ih * HNT, ih * HNT + HNT):
                    vj = vjts_by_it[it]
                    # ctx PV accumulation in [:, :128] and the softmax
                    # denominator (ones-matmul row sums) in [:, 128:129],
                    # sharing one PSUM bank: the first matmul's start=True
                    # clears the whole bank, later ones overwrite-or-
                    # accumulate via the per-element has_written bits.
                    ctxps = ppb.tile([128, 132], f32, tag="acc", bufs=2,
                                     name=f"cps_{hl}_{it}")
                    slots = []
                    for jt in vj:
                        pt, bse = pT[(jt, 1 if it >= HNT else 0)]
                        off = (it - bse) * 128
                        slots.append(pt[:, off:off + 128])
                    for idx, sl in enumerate(slots):
                        nc.tensor.matmul(ctxps[:, :128], lhsT=sl,
                                         rhs=v_sb[:, vj[idx], :],
                                         start=(idx == 0), stop=False,
                                         skip_group_check=True)
                    for idx, sl in enumerate(slots):
                        nc.tensor.matmul(ctxps[:, 128:129], lhsT=sl,
                                         rhs=ones_sb,
                                         start=False,
                                         stop=(idx == len(slots) - 1),
                                         skip_group_check=True)
                    rec = pb.tile([128, 1], f32, tag="rec", bufs=3,
                                  name=f"rc_{hl}_{it}")
                    nc.vector.reciprocal(out=rec, in_=ctxps[:, 128:129])
                    cti = pb.tile([128, 128], bf16, tag="cti", bufs=8,
                                  name=f"ci_{hl}_{it}")
                    nc.vector.tensor_scalar_mul(cti, ctxps[:, :128], rec)
                    ctis.append(cti)
                    for _ in range(per_it):
                        if side:
                            side.pop(0)()
                    if it % 4 == 3 and it >= 7:
                        flush(it // 4 - 1)
                for em in side:
                    em()
                if ih == 1:
                    flush(3)

            def interleave(*lists):
                out = []
                k = 0
                lists = [list(x) for x in lists]
                while any(lists):
                    if lists[k % len(lists)]:
                        out.append(lists[k % len(lists)].pop(0))
                    k += 1
                return out

            # software pipeline over (head, i-half) windows
            qr, kr, em0 = rope(0)
            for em in em0:
                em()
            v_sb, vt0 = vtrans(0)
            for em in vt0:
                em()
            heads = {0: (qr, kr, v_sb)}
            pTs = {0: {}}
            for em in scores_half(0, 0, qr, kr, pTs[0]):
                em()
            ctis_h = {hl: [] for hl in range(HPC)}
            for hl in range(HPC):
                qr, kr, v_sb = heads[hl]
                side0 = [scores_half(hl, 1, qr, kr, pTs[hl])]
                if hl + 1 < HPC:
                    nqr, nkr, rope_em = rope(hl + 1)
                    nv_sb, vt_em = vtrans(hl + 1)
                    heads[hl + 1] = (nqr, nkr, nv_sb)
                    pTs[hl + 1] = {}
                    side0.append(rope_em)
                pv_half(hl, 0, pTs[hl], v_sb, ctis_h[hl],
                        interleave(*side0))
                side1 = []
                if hl + 1 < HPC:
                    side1 = interleave(
                        scores_half(hl + 1, 0, nqr, nkr, pTs[hl + 1]),
                        vt_em)
                if hl == HPC - 2:
                    for nb in range(3):
                        load_wd(nb)
                pv_half(hl, 1, pTs[hl], v_sb, ctis_h[hl], side1)

        # ---------------- Stage C: column-split row-parallel dense -------
        with ExitStack() as sc:
            ppc = sc.enter_context(tc.tile_pool(name="dense_psum", bufs=1,
                                                space="PSUM"))
            for nb in range(32):
                wd = wd_tiles.pop(nb)
                ob = pc.tile([128, S], bf16, tag="ob", bufs=4,
                             name=f"ob_{nb}")
                for ci, (c0, c1, e) in enumerate(out_chunks):
                    w = c1 - c0
                    opf = ppc.tile([128, 512], f32, tag="ops", bufs=6,
                                   name=f"o_{nb}_{c0}")
                    ops = opf[:, :w]
                    for dt_ in range(HPC):
                        nc.tensor.matmul(ops, lhsT=wd[:, e, dt_, :],
                                         rhs=ctxT_sb[dt_][:, c0:c1],
                                         start=(dt_ == 0),
                                         stop=(dt_ == HPC - 1))
                    if (nb + ci) % 2 == 0:
                        nc.vector.tensor_copy(out=ob[:, c0:c1], in_=ops)
                    else:
                        nc.scalar.copy(out=ob[:, c0:c1], in_=ops)
                nc.gpsimd.dma_start(out=outT[nb, :, :], in_=ob)
                if nb + 3 < 32:
                    load_wd(nb + 3)

    nc.finalize()
    return nc


def _host_prep(inputs):
    import ml_dtypes
    bf = ml_dtypes.bfloat16

    hs = np.asarray(inputs["hidden_states"], dtype=np.float32).reshape(S, H)
    tt = np.asarray(inputs["token_type_ids"]).reshape(S)
    pos = np.asarray(inputs["position_ids"]).reshape(S).astype(np.int64)
    am = np.asarray(inputs["attention_mask"], dtype=np.float32).reshape(
        np.asarray(inputs["attention_mask"]).shape[-2], -1)[:S, :S]
    wv_qkv = np.asarray(inputs["wv_qkv"], dtype=np.float32)
    wl_qkv = np.asarray(inputs["wl_qkv"], dtype=np.float32)
    wv_dense = np.asarray(inputs["wv_dense"], dtype=np.float32)
    wl_dense = np.asarray(inputs["wl_dense"], dtype=np.float32)

    # routing mask: vision iff tt[i]==1 and tt[i+1]==1; last position language
    core = (tt[:-1] == 1) & (tt[1:] == 1)
    vmb = np.concatenate([core, [False]])

    # sort tokens: language first (stable) -> expert is a column split at L
    perm = np.argsort(vmb, kind="stable")
    L = int((~vmb).sum())
    hs_p = hs[perm]
    pos_p = pos[perm]
    am_p = np.ascontiguousarray(am[np.ix_(perm, perm)])

    # hsT tiled [128(p), 2(th), 32(kt), 1024(t)]
    hsb = np.ascontiguousarray(
        hs_p.astype(bf).reshape(2, 1024, 32, 128).transpose(3, 0, 2, 1))

    inv_freq = 1.0 / (ROPE_BASE ** (np.arange(0, HD, 2, dtype=np.float32) / HD))
    t = np.arange(S, dtype=np.float32)
    emb = np.concatenate([np.outer(t, inv_freq)] * 2, axis=-1)  # [S, HD]
    ss = np.float32(np.sqrt(1.0 / np.sqrt(HD)))
    cosT = np.ascontiguousarray((np.cos(emb) * ss)[pos_p].T.astype(bf))
    sinT = np.ascontiguousarray((np.sin(emb) * ss)[pos_p].T.astype(bf))

    # per-(i-tile, j-tile) mask status: 0=all-zero, 1=mixed, 2=all-masked
    mask_info = []
    for it in range(NT):
        row = []
        for jt in range(NT):
            blk = am_p[it * 128:(it + 1) * 128, jt * 128:(jt + 1) * 128]
            if blk.max() < -1e8:
                row.append(2)
            elif blk.min() == 0.0 and blk.max() == 0.0:
                row.append(0)
            else:
                row.append(1)
        mask_info.append(tuple(row))
    mask_info = tuple(mask_info)

    # packed binary keep-masks, transposed: mpk[p(j), b, c(i)]
    mblocks = []
    for jt in range(NT):
        for it in range(NT):
            if mask_info[it][jt] == 1:
                blk = am_p[it * 128:(it + 1) * 128,
                           jt * 128:(jt + 1) * 128]
                mblocks.append((blk == 0.0).T.astype(bf))
    if mblocks:
        mpk = np.ascontiguousarray(np.stack(mblocks, axis=1))
    else:
        mpk = np.zeros((128, 1, 128), dtype=bf)

    in_maps = []
    for cid in range(NCORES):
        heads = range(HPC * cid, HPC * (cid + 1))
        blocks = [[], []]
        for h in heads:
            for part in range(3):  # q, k, v
                col0 = part * H + h * HD
                for ei, W in enumerate((wv_qkv, wl_qkv)):
                    blocks[ei].append(
                        W[:, col0:col0 + HD].astype(bf)
                        .reshape(32, 128, 128).transpose(1, 0, 2))
        wqkv_c = np.ascontiguousarray(
            np.stack([np.stack(blocks[0]), np.stack(blocks[1])]))
        # -> [2, NBLK, 128(p), 32(kt), 128(c)]
        r0, r1 = HPC * cid * HD, HPC * (cid + 1) * HD
        wd = np.stack([wv_dense[r0:r1], wl_dense[r0:r1]])  # [2,512,4096]
        wdense_c = np.ascontiguousarray(
            wd.astype(bf).reshape(2, HPC, 128, 32, 128)
            .transpose(3, 2, 0, 1, 4))  # [32(nb),128(p),2,HPC,128(c)]
        im = {
            "hsT": hsb,
            "wqkv": wqkv_c,
            "wdense": wdense_c,
            "cosT": cosT,
            "sinT": sinT,
            "mpk": mpk,
        }
        in_maps.append(im)
    return (L, mask_info), perm, in_maps


PROFILE = False
LAST_EXEC_NS = None
LAST_RESULTS = None


def kernel(**inputs):
    global LAST_EXEC_NS, LAST_RESULTS
    from concourse.bass_utils import run_bass_kernel_spmd

    key, perm, in_maps = _host_prep(inputs)
    if key not in _CACHE:
        _CACHE[key] = _build(*key)
    nc = _CACHE[key]
    kw = {"trace": True} if PROFILE else {}
    res = run_bass_kernel_spmd(nc, in_maps, core_ids=list(range(NCORES)), **kw)
    LAST_EXEC_NS = res.exec_time_ns
    LAST_RESULTS = res
    acc = np.zeros((H, S), dtype=np.float32)
    for r in res.results:
        acc += np.asarray(r["outT"], dtype=np.float32).reshape(H, S)
    out = np.empty((S, H), dtype=np.float32)
    out[perm] = acc.T
    return np.ascontiguousarray(out).reshape(B, S, H)


# revision 34
# speedup vs baseline: 1.0054x; 1.0054x over previous
"""CogVLM vision-expert attention on 8 Trainium2 NeuronCores.

Sharding: tensor-parallel over heads (4 heads per core). Each core gets
- replicated: hidden_states (bf16, [128,2,32,1024] tiled), RoPE tables,
  packed binary key-mask blocks
- sharded:    QKV weight columns + dense weight rows for its 4 heads
Host sorts tokens language-first, so expert routing becomes an exact
column split at L (no dual-expert compute, no predicated selects).
Attention computes transposed scores s^T[j,i] per key-block, exps them
PSUM->SBUF, applies the 0/1 mask multiplicatively, row-sums via
ones-matmuls on the PE, and folds 1/sum into the PSUM->SBUF context
copy.  Everything on the matmul path is bf16 (f32 PSUM accumulate).
The host sums the 8 bf16 row-parallel dense partials.

Self-contained: hardcodes all shapes; only needs numpy + ml_dtypes +
concourse (on sys.path in this container).
"""

import numpy as np

B, S, H, NH = 1, 2048, 4096, 32
HD = H // NH          # 128
NCORES = 8
HPC = NH // NCORES    # 4 heads per core
NBLK = 3 * HPC        # 12 qkv col-blocks of 128 per core
NT = S // 128         # 16 token blocks
ROPE_BASE = 10000.0

_CACHE = {}


def _chunks(lo, hi, L):
    """Split [lo,hi) at multiples of 512 and at L -> [(c0,c1,expert)].
    expert 1 = language (tokens < L), 0 = vision."""
    pts = {lo, hi}
    pts.update(c for c in range(0, S + 1, 512) if lo < c < hi)
    if lo < L < hi:
        pts.add(L)
    pts = sorted(pts)
    return [(pts[i], pts[i + 1], 1 if pts[i + 1] <= L else 0)
            for i in range(len(pts) - 1)]


def _build(L, mask_info):
    import concourse.bass as bass
    import concourse.mybir as mybir
    import concourse.tile as tile
    from concourse import bacc
    from contextlib import ExitStack
    import ml_dtypes

    dt = mybir.dt
    f32, bf16 = dt.float32, dt.bfloat16
    AF = mybir.ActivationFunctionType

    # ---- derived compile-time structure from the mask ----
    # mask_info[it][jt]: 0 = all-allowed, 1 = mixed, 2 = all-masked
    mix_idx = {}
    for jt in range(NT):
        for it in range(NT):
            if mask_info[it][jt] == 1:
                mix_idx[(jt, it)] = len(mix_idx)
    NMIX = len(mix_idx)

    HNT = NT // 2
    # per (jt, i-half): valid-it segments (<=4 blocks) and mixed-it runs
    segs_by = {}      # (jt, ih) -> [(i0b, i1b)]
    mixruns_by = {}   # (jt, ih) -> [(a0b, a1b, mstart)]
    for jt in range(NT):
        for ih in range(2):
            lo, hi = ih * HNT, ih * HNT + HNT
            valid = [it for it in range(lo, hi) if mask_info[it][jt] != 2]
            segs = []
            k = 0
            while k < len(valid):
                j0 = k
                while (k + 1 < len(valid) and valid[k + 1] == valid[k] + 1
                       and k + 1 - j0 < 8):
                    k += 1
                segs.append((valid[j0], valid[k] + 1))
                k += 1
            segs_by[(jt, ih)] = segs
            runs = []
            k = 0
            mixed = [it for it in range(lo, hi) if mask_info[it][jt] == 1]
            while k < len(mixed):
                j0 = k
                while k + 1 < len(mixed) and mixed[k + 1] == mixed[k] + 1:
                    k += 1
                runs.append((mixed[j0], mixed[k] + 1,
                             mix_idx[(jt, mixed[j0])]))
                k += 1
            mixruns_by[(jt, ih)] = runs
    vjts_by_it = [[jt for jt in range(NT) if mask_info[it][jt] != 2]
                  for it in range(NT)]

    nc = bacc.Bacc("TRN2", target_bir_lowering=False, debug=False)

    hsT = nc.dram_tensor("hsT", [128, 2, 32, 1024], bf16, kind="ExternalInput")
    wqkv = nc.dram_tensor("wqkv", [2, NBLK, 128, 32, 128], bf16,
                          kind="ExternalInput")
    wdense = nc.dram_tensor("wdense", [32, 128, 2, HPC, 128], bf16,
                            kind="ExternalInput")
    cosT = nc.dram_tensor("cosT", [HD, S], bf16, kind="ExternalInput")
    sinT = nc.dram_tensor("sinT", [HD, S], bf16, kind="ExternalInput")
    mpk = nc.dram_tensor("mpk", [128, max(NMIX, 1), 128], bf16,
                         kind="ExternalInput")
    outT = nc.dram_tensor("outT", [32, 128, S], bf16, kind="ExternalOutput")

    eye_bf16 = nc.inline_tensor(np.eye(128, dtype=ml_dtypes.bfloat16),
                                "eye_bf16")
    RT_np = np.zeros((128, 128), dtype=np.float32)
    for j in range(64):
        RT_np[j, j + 64] = 1.0
        RT_np[j + 64, j] = -1.0
    RT_t = nc.inline_tensor(RT_np.astype(ml_dtypes.bfloat16), "RT")
    ones_t = nc.inline_tensor(np.ones((128, 1), dtype=ml_dtypes.bfloat16),
                              "ones")

    qkv_chunks = {th: _chunks(th * 1024, th * 1024 + 1024, L)
                  for th in range(2)}
    out_chunks = _chunks(0, S, L)

    with tile.TileContext(nc) as tc, ExitStack() as top:
        singles = top.enter_context(tc.tile_pool(name="singles", bufs=1))

        ident16 = singles.tile([128, 128], bf16)
        nc.gpsimd.dma_start(out=ident16, in_=eye_bf16[:, :])
        RT_sb = singles.tile([128, 128], bf16)
        nc.gpsimd.dma_start(out=RT_sb, in_=RT_t[:, :])
        ones_sb = singles.tile([128, 1], bf16)
        nc.gpsimd.dma_start(out=ones_sb, in_=ones_t[:, :])
        cos_sb = singles.tile([HD, S], bf16)
        nc.gpsimd.dma_start(out=cos_sb, in_=cosT[:, :])
        sin_sb = singles.tile([HD, S], bf16)
        nc.gpsimd.dma_start(out=sin_sb, in_=sinT[:, :])
        nbias = singles.tile([128, 1], f32)
        nc.vector.memset(nbias, -24.0)

        qkv_pool = top.enter_context(tc.tile_pool(name="qkv", bufs=1))
        qkv_sb = [qkv_pool.tile([128, S], bf16, tag="qkv", bufs=NBLK,
                                name=f"qkv_{nb}") for nb in range(NBLK)]
        ctx_pool = top.enter_context(tc.tile_pool(name="ctx", bufs=1))
        ctxT_sb = [ctx_pool.tile([128, S], bf16, tag="ctxT", bufs=HPC,
                                 name=f"ctxT_{hl}") for hl in range(HPC)]

        # ---------------- Stage A: column-split QKV projection ----------
        with ExitStack() as sa:
            pa = sa.enter_context(tc.tile_pool(name="qkv_sbuf", bufs=1))
            ppa = sa.enter_context(tc.tile_pool(name="qkv_psum", bufs=1,
                                                space="PSUM"))
            ncopy = 0
            for th in range(2):
                t0 = th * 1024
                experts = sorted({e for _, _, e in qkv_chunks[th]})
                hk = []
                for i in range(8):
                    hki = pa.tile([128, 4, 1024], bf16, tag="hsT", bufs=10,
                                  name=f"hsT_{th}_{i}")
                    nc.gpsimd.dma_start(out=hki,
                                        in_=hsT[:, th, 4 * i:4 * i + 4, :])
                    hk.append(hki)
                for nb in range(NBLK):
                    wb = {}
                    for e in experts:
                        wbe = pa.tile([128, 32, 128], bf16, tag=f"w{e}",
                                      bufs=3, name=f"w_{th}_{nb}_{e}")
                        nc.sync.dma_start(out=wbe, in_=wqkv[e, nb, :, :, :])
                        wb[e] = wbe
                    for c0, c1, e in qkv_chunks[th]:
                        w = c1 - c0
                        psf = ppa.tile([128, 512], f32, tag="ps", bufs=4,
                                       name=f"ps_{th}_{nb}_{c0}")
                        ps = psf[:, :w]
                        for kt in range(32):
                            nc.tensor.matmul(
                                ps,
                                lhsT=wb[e][:, kt, :],
                                rhs=hk[kt // 4][:, kt % 4,
                                                c0 - t0:c1 - t0],
                                start=(kt == 0), stop=(kt == 31),
                            )
                        dst = qkv_sb[nb][:, c0:c1]
                        if ncopy % 2 == 0:
                            nc.vector.tensor_copy(out=dst, in_=ps)
                        else:
                            nc.scalar.copy(out=dst, in_=ps)
                        ncopy += 1

        # ---------------- Stage B: per-head attention -------------------
        pc = top.enter_context(tc.tile_pool(name="dense_sbuf", bufs=1))
        wd_tiles = {}

        def load_wd(nb):
            wd = pc.tile([128, 2, HPC, 128], bf16, tag="wd", bufs=3,
                         name=f"wd_{nb}")
            nc.sync.dma_start(out=wd, in_=wdense[nb, :, :, :, :])
            wd_tiles[nb] = wd

        with ExitStack() as sb:
            pb = sb.enter_context(tc.tile_pool(name="att_sbuf", bufs=1))
            ppb = sb.enter_context(tc.tile_pool(name="att_psum", bufs=1,
                                                space="PSUM"))
            mq = pb.tile([128, max(NMIX, 1), 128], bf16, tag="mq", bufs=1)
            nc.gpsimd.dma_start(out=mq, in_=mpk[:, :, :])

            def rope(hl):
                # RoPE: x' = x*cos + (R @ x)*sin (scale folded into tables).
                # Returns per-chunk emitters so callers can interleave them
                # under other PE work (the DVE muls are the long pole).
                qr = pb.tile([128, S], bf16, tag="qr", bufs=2,
                             name=f"qr_{hl}")
                kr = pb.tile([128, S], bf16, tag="kr", bufs=2,
                             name=f"kr_{hl}")

                def chunk(xT, xr, ch):
                    def emit():
                        cs = slice(ch * 1024, ch * 1024 + 1024)
                        rps = ppb.tile([128, 1024], f32, tag="mm", bufs=3,
                                       name=f"rot_{hl}_{ch}")
                        for s in range(2):
                            nc.tensor.matmul(
                                rps[:, s * 512:s * 512 + 512],
                                lhsT=RT_sb,
                                rhs=xT[:, ch * 1024 + s * 512:
                                       ch * 1024 + s * 512 + 512],
                                start=True, stop=True)
                        t1 = pb.tile([128, 1024], bf16, tag="ropet", bufs=3,
                                     name=f"rt_{hl}_{ch}")
                        nc.vector.tensor_mul(out=t1, in0=rps,
                                             in1=sin_sb[:, cs])
                        nc.vector.tensor_mul(out=xr[:, cs], in0=xT[:, cs],
                                             in1=cos_sb[:, cs])
                        nc.vector.tensor_add(out=xr[:, cs], in0=xr[:, cs],
                                             in1=t1)
                    return emit

                emitters = [chunk(xT, xr, ch)
                            for xT, xr in ((qkv_sb[3 * hl], qr),
                                           (qkv_sb[3 * hl + 1], kr))
                            for ch in range(2)]
                return qr, kr, emitters

            def vtrans(hl):
                # 4 transposes packed per PSUM bank (start=True on the first
                # clears the whole bank), then one wide DVE copy each.
                v_sb = pb.tile([128, NT, 128], bf16, tag="vsb", bufs=2,
                               name=f"v_{hl}")
                vT = qkv_sb[3 * hl + 2]

                def group(g):
                    def emit():
                        vtpf = ppb.tile([128, 1024], f32, tag="mm", bufs=3,
                                        name=f"vt_{hl}_{g}")
                        vtp = vtpf.bitcast(bf16)[:, :512]
                        for k in range(4):
                            jt = 4 * g + k
                            nc.tensor.matmul(
                                vtp[:, k * 128:(k + 1) * 128],
                                lhsT=vT[:, jt * 128:(jt + 1) * 128],
                                rhs=ident16, is_transpose=True,
                                start=(k == 0), stop=(k == 3),
                                skip_group_check=True,
                            )
                        nc.vector.tensor_copy(
                            out=v_sb[:, 4 * g:4 * g + 4, :], in_=vtp)
                    return emit

                return v_sb, [group(g) for g in range(4)]

            def scores_half(hl, ih, qr, kr, pT):
                # per-jt closures: transposed scores + exp + mult. mask for
                # query blocks in i-half ih; results land in pT[(jt, ih)].
                def one(jt):
                    def emit():
                        segs = segs_by[(jt, ih)]
                        base = segs[0][0]
                        width = (segs[-1][1] - base) * 128
                        pt = pb.tile([128, width], bf16,
                                     tag=f"pT{jt}_{ih}", bufs=1,
                                     name=f"pT_{hl}_{jt}_{ih}")
                        pT[(jt, ih)] = (pt, base)
                        for i0b, i1b in segs:
                            w = (i1b - i0b) * 128
                            spf = ppb.tile([128, 1024], f32, tag="mm",
                                           bufs=3,
                                           name=f"sp_{hl}_{jt}_{i0b}")
                            sp = spf[:, :w]
                            for s in range(0, w, 512):
                                nc.tensor.matmul(
                                    sp[:, s:min(w, s + 512)],
                                    lhsT=kr[:, jt * 128:(jt + 1) * 128],
                                    rhs=qr[:, i0b * 128 + s:
                                           i0b * 128 + min(w, s + 512)],
                                    start=True, stop=True,
                                )
                            off = (i0b - base) * 128
                            # logits are O(10) at this input scale;
                            # exp(x-24) cannot overflow and softmax is
                            # shift-invariant.
                            nc.scalar.activation(
                                out=pt[:, off:off + w], in_=sp,
                                func=AF.Exp, bias=nbias, scale=1.0,
                            )
                        for a0, a1, m0 in mixruns_by[(jt, ih)]:
                            w = (a1 - a0) * 128
                            off = (a0 - base) * 128
                            nc.vector.tensor_mul(
                                out=pt[:, off:off + w],
                                in0=pt[:, off:off + w],
                                in1=mq[:, m0:m0 + (a1 - a0), :],
                            )
                    return emit

                return [one(jt) for jt in range(NT) if segs_by[(jt, ih)]]

            def pv_half(hl, ih, pT, v_sb, ctis, side=()):
                # per-query-block: denominators, PV, DVE scale; transposes
                # batched 4-per-bank, deferred one group so the chain hides
                # under the next group's PV matmuls.  `side` closures (next
                # half/head's scores + rope) are drained between blocks.
                side = list(side)

                def flush(g):
                    ctpf = ppb.tile([128, 1024], f32, tag="mm", bufs=3,
                                    name=f"ct_{hl}_{g}")
                    ctp = ctpf.bitcast(bf16)[:, :512]
                    for k in range(4):
                        nc.tensor.matmul(
                            ctp[:, k * 128:(k + 1) * 128],
                            lhsT=ctis[4 * g + k], rhs=ident16,
                            is_transpose=True,
                            start=(k == 0), stop=(k == 3),
                            skip_group_check=True,
                        )
                    nc.vector.tensor_copy(
                        out=ctxT_sb[hl][:, g * 512:(g + 1) * 512],
                        in_=ctp)

                per_it = -(-len(side) // HNT)
                for it in range(ih * HNT, ih * HNT + HNT):
                    vj = vjts_by_it[it]
                    # ctx PV accumulation in [:, :128] and the softmax
                    # denominator (ones-matmul row sums) in [:, 128:129],
                    # sharing one PSUM bank: the first matmul's start=True
                    # clears the whole bank, later ones overwrite-or-
                    # accumulate via the per-element has_written bits.
                    ctxps = ppb.tile([128, 132], f32, tag="acc", bufs=2,
                                     name=f"cps_{hl}_{it}")
                    slots = []
                    for jt in vj:
                        pt, bse = pT[(jt, 1 if it >= HNT else 0)]
                        off = (it - bse) * 128
                        slots.append(pt[:, off:off + 128])
                    for idx, sl in enumerate(slots):
                        nc.tensor.matmul(ctxps[:, :128], lhsT=sl,
                                         rhs=v_sb[:, vj[idx], :],
                                         start=(idx == 0), stop=False,
                                         skip_group_check=True)
                    for idx, sl in enumerate(slots):
                        nc.tensor.matmul(ctxps[:, 128:129], lhsT=sl,
                                         rhs=ones_sb,
                                         start=False,
                                         stop=(idx == len(slots) - 1),
                                         skip_group_check=True)
                    rec = pb.tile([128, 1], f32, tag="rec", bufs=3,
                                  name=f"rc_{hl}_{it}")
                    nc.vector.reciprocal(out=rec, in_=ctxps[:, 128:129])
                    cti = pb.tile([128, 128], bf16, tag="cti", bufs=8,
                                  name=f"ci_{hl}_{it}")
                    nc.vector.tensor_scalar_mul(cti, ctxps[:, :128], rec)
                    ctis.append(cti)
                    for _ in range(per_it):
                        if side:
                            side.pop(0)()
                    if it % 4 == 3 and it >= 7:
                        flush(it // 4 - 1)
                for em in side:
                    em()
                if ih == 1:
                    flush(3)

            def interleave(*lists):
                out = []
                k = 0
                lists = [list(x) for x in lists]
                while any(lists):
                    if lists[k % len(lists)]:
                        out.append(lists[k % len(lists)].pop(0))
                    k += 1
                return out

            # software pipeline over (head, i-half) windows
            # prelude: v transposes first give the PE work while the first
            # RoPE DVE chains drain; scores follow as soon as qr/kr land.
            qr, kr, em0 = rope(0)
            v_sb, vt0 = vtrans(0)
            em0[0]()
            em0[2]()
            for em in interleave(vt0, [em0[1], em0[3]]):
                em()
            heads = {0: (qr, kr, v_sb)}
            pTs = {0: {}}
            for em in scores_half(0, 0, qr, kr, pTs[0]):
                em()
            ctis_h = {hl: [] for hl in range(HPC)}
            for hl in range(HPC):
                qr, kr, v_sb = heads[hl]
                side0 = [scores_half(hl, 1, qr, kr, pTs[hl])]
                if hl + 1 < HPC:
                    nqr, nkr, rope_em = rope(hl + 1)
                    nv_sb, vt_em = vtrans(hl + 1)
                    heads[hl + 1] = (nqr, nkr, nv_sb)
                    pTs[hl + 1] = {}
                    side0.append(rope_em)
                pv_half(hl, 0, pTs[hl], v_sb, ctis_h[hl],
                        interleave(*side0))
                side1 = []
                if hl + 1 < HPC:
                    side1 = interleave(
                        scores_half(hl + 1, 0, nqr, nkr, pTs[hl + 1]),
                        vt_em)
                if hl == HPC - 2:
                    for nb in range(3):
                        load_wd(nb)
                pv_half(hl, 1, pTs[hl], v_sb, ctis_h[hl], side1)

        # ---------------- Stage C: column-split row-parallel dense -------
        with ExitStack() as sc:
            ppc = sc.enter_context(tc.tile_pool(name="dense_psum", bufs=1,
                                                space="PSUM"))
            for nb in range(32):
                wd = wd_tiles.pop(nb)
                ob = pc.tile([128, S], bf16, tag="ob", bufs=4,
                             name=f"ob_{nb}")
                for ci, (c0, c1, e) in enumerate(out_chunks):
                    w = c1 - c0
                    opf = ppc.tile([128, 512], f32, tag="ops", bufs=6,
                                   name=f"o_{nb}_{c0}")
                    ops = opf[:, :w]
                    for dt_ in range(HPC):
                        nc.tensor.matmul(ops, lhsT=wd[:, e, dt_, :],
                                         rhs=ctxT_sb[dt_][:, c0:c1],
                                         start=(dt_ == 0),
                                         stop=(dt_ == HPC - 1))
                    if (nb + ci) % 2 == 0:
                        nc.vector.tensor_copy(out=ob[:, c0:c1], in_=ops)
                    else:
                        nc.scalar.copy(out=ob[:, c0:c1], in_=ops)
                nc.gpsimd.dma_start(out=outT[nb, :, :], in_=ob)
                if nb + 3 < 32:
                    load_wd(nb + 3)

    nc.finalize()
    return nc


def _host_prep(inputs):
    import ml_dtypes
    bf = ml_dtypes.bfloat16

    hs = np.asarray(inputs["hidden_states"], dtype=np.float32).reshape(S, H)
    tt = np.asarray(inputs["token_type_ids"]).reshape(S)
    pos = np.asarray(inputs["position_ids"]).reshape(S).astype(np.int64)
    am = np.asarray(inputs["attention_mask"], dtype=np.float32).reshape(
        np.asarray(inputs["attention_mask"]).shape[-2], -1)[:S, :S]
    wv_qkv = np.asarray(inputs["wv_qkv"], dtype=np.float32)
    wl_qkv = np.asarray(inputs["wl_qkv"], dtype=np.float32)
    wv_dense = np.asarray(inputs["wv_dense"], dtype=np.float32)
    wl_dense = np.asarray(inputs["wl_dense"], dtype=np.float32)

    # routing mask: vision iff tt[i]==1 and tt[i+1]==1; last position language
    core = (tt[:-1] == 1) & (tt[1:] == 1)
    vmb = np.concatenate([core, [False]])

    # sort tokens: language first (stable) -> expert is a column split at L
    perm = np.argsort(vmb, kind="stable")
    L = int((~vmb).sum())
    hs_p = hs[perm]
    pos_p = pos[perm]
    am_p = np.ascontiguousarray(am[np.ix_(perm, perm)])

    # hsT tiled [128(p), 2(th), 32(kt), 1024(t)]
    hsb = np.ascontiguousarray(
        hs_p.astype(bf).reshape(2, 1024, 32, 128).transpose(3, 0, 2, 1))

    inv_freq = 1.0 / (ROPE_BASE ** (np.arange(0, HD, 2, dtype=np.float32) / HD))
    t = np.arange(S, dtype=np.float32)
    emb = np.concatenate([np.outer(t, inv_freq)] * 2, axis=-1)  # [S, HD]
    ss = np.float32(np.sqrt(1.0 / np.sqrt(HD)))
    cosT = np.ascontiguousarray((np.cos(emb) * ss)[pos_p].T.astype(bf))
    sinT = np.ascontiguousarray((np.sin(emb) * ss)[pos_p].T.astype(bf))

    # per-(i-tile, j-tile) mask status: 0=all-zero, 1=mixed, 2=all-masked
    mask_info = []
    for it in range(NT):
        row = []
        for jt in range(NT):
            blk = am_p[it * 128:(it + 1) * 128, jt * 128:(jt + 1) * 128]
            if blk.max() < -1e8:
                row.append(2)
            elif blk.min() == 0.0 and blk.max() == 0.0:
                row.append(0)
            else:
                row.append(1)
        mask_info.append(tuple(row))
    mask_info = tuple(mask_info)

    # packed binary keep-masks, transposed: mpk[p(j), b, c(i)]
    mblocks = []
    for jt in range(NT):
        for it in range(NT):
            if mask_info[it][jt] == 1:
                blk = am_p[it * 128:(it + 1) * 128,
                           jt * 128:(jt + 1) * 128]
                mblocks.append((blk == 0.0).T.astype(bf))
    if mblocks:
        mpk = np.ascontiguousarray(np.stack(mblocks, axis=1))
    else:
        mpk = np.zeros((128, 1, 128), dtype=bf)

    in_maps = []
    for cid in range(NCORES):
        heads = range(HPC * cid, HPC * (cid + 1))
        blocks = [[], []]
        for h in heads:
            for part in range(3):  # q, k, v
                col0 = part * H + h * HD
                for ei, W in enumerate((wv_qkv, wl_qkv)):
                    blocks[ei].append(
                        W[:, col0:col0 + HD].astype(bf)
                        .reshape(32, 128, 128).transpose(1, 0, 2))
        wqkv_c = np.ascontiguousarray(
            np.stack([np.stack(blocks[0]), np.stack(blocks[1])]))
        # -> [2, NBLK, 128(p), 32(kt), 128(c)]
        r0, r1 = HPC * cid * HD, HPC * (cid + 1) * HD
        wd = np.stack([wv_dense[r0:r1], wl_dense[r0:r1]])  # [2,512,4096]
        wdense_c = np.ascontiguousarray(
            wd.astype(bf).reshape(2, HPC, 128, 32, 128)
            .transpose(3, 2, 0, 1, 4))  # [32(nb),128(p),2,HPC,128(c)]
        im = {
            "hsT": hsb,
            "wqkv": wqkv_c,
            "wdense": wdense_c,
            "cosT": cosT,
            "sinT": sinT,
            "mpk": mpk,
        }
        in_maps.append(im)
    return (L, mask_info), perm, in_maps


PROFILE = False
LAST_EXEC_NS = None
LAST_RESULTS = None


def kernel(**inputs):
    global LAST_EXEC_NS, LAST_RESULTS
    from concourse.bass_utils import run_bass_kernel_spmd

    key, perm, in_maps = _host_prep(inputs)
    if key not in _CACHE:
        _CACHE[key] = _build(*key)
    nc = _CACHE[key]
    kw = {"trace": True} if PROFILE else {}
    res = run_bass_kernel_spmd(nc, in_maps, core_ids=list(range(NCORES)), **kw)
    LAST_EXEC_NS = res.exec_time_ns
    LAST_RESULTS = res
    acc = np.zeros((H, S), dtype=np.float32)
    for r in res.results:
        acc += np.asarray(r["outT"], dtype=np.float32).reshape(H, S)
    out = np.empty((S, H), dtype=np.float32)
    out[perm] = acc.T
    return np.ascontiguousarray(out).reshape(B, S, H)
